# revision 12
# baseline (speedup 1.0000x reference)
"""GATv2 multi-head attention kernel for Trainium2 (8 NeuronCores).

Problem: nn_GATv2MHA  (b=4, n=512, input_dim=128, 8 heads x head_dim 16)
  g_l = einsum('bni,hid->hbnd', h, Wl); g_r likewise
  e = leaky_relu(g_l[:,:,:,None,:] + g_r[:,:,None,:,:], 0.2)
  scores = einsum('hbijd,hd->hbij', e, Wak);  attn = softmax(scores, -1)
  out = relu(einsum('hbij,hbjd->bihd', attn, g_r)).reshape(b, n, -1)

Sharding: data-parallel over (batch, token-half): core c handles batch c//2,
token rows [(c%2)*256, (c%2)*256+256).  No collectives.  The j (key) axis is
permuted per-core so the core's own half comes first — softmax over j is
permutation invariant, which lets one hT input serve as both the full key set
and the query slice (cols 0:256).

Math: LeakyRelu(x) = 0.2*x + 0.8*relu(x); the 0.2*u_i row-constant cancels in
softmax, so  scores'[q_row, j] = 0.8 * sum_d a_hd relu(gl_hid + gr_hjd)
                               + 0.2 * v_hj          (v_hj = sum_d a_hd gr_hjd)
Per i-row one tensor_scalar(add,max) makes X = relu(gl_col + grT) (spread
across DVE/ACT/GPSIMD), and one PE matmul with the shared m=8 weight A8
contracts d per head into S rows (i2,h) = i2*8+h; the Wv matmul adds 0.2*v_j.
Softmax: no max subtraction — scores are bounded (|s| < ~30) so P = exp(S) in
bf16 cannot overflow; Z comes from a ones-column appended to gr_cat so the
out-projection produces it for free (column 128)."""

import functools
import os

import numpy as np
import ml_dtypes

N_HEADS = 8
INPUT_DIM = 128
HEAD_DIM = 16
B = 4
N = 512
N_CORES = 8
HALF = N // 2          # token rows per core
BLK = 16               # i-rows per block (16 i x 8 heads = 128 S rows)
NBLK = HALF // BLK     # 16 blocks per core

# ---- tunables (overridable via env for experiments) ----------------------
POOL_QS = int(os.environ.get("GAT_POOL_QS", "4"))   # X-ops on GpSimd (late q)
ACT_QS = int(os.environ.get("GAT_ACT_QS", "3"))     # X-ops on ScalarE (last q)
PT_ASSIGN = os.environ.get("GAT_PT", "dada")        # 4 PT copies: d/a engines
                                                    # (GPSIMD cannot read PSUM)
XBUFS = int(os.environ.get("GAT_XBUFS", "10"))
SBUFS = int(os.environ.get("GAT_SBUFS", "4"))       # PSUM score banks
PBUFS = int(os.environ.get("GAT_PBUFS", "2"))
DT_P = os.environ.get("GAT_DT_P", "bf16")           # P / PT / ident dtype
DT_GR = os.environ.get("GAT_DT_GR", "f16")          # grT / gr_cat dtype
TAIL2_Q = int(os.environ.get("GAT_TAIL2_Q", "10"))  # q at which recip/relu emit


def _mydt(s):
    import concourse.mybir as mybir
    return {"bf16": mybir.dt.bfloat16, "f16": mybir.dt.float16,
            "f32": mybir.dt.float32}[s]


def _npdt(s):
    return {"bf16": ml_dtypes.bfloat16, "f16": np.float16,
            "f32": np.float32}[s]


def build_program():
    """Build + compile the (identical-across-cores) Bass program."""
    import concourse.bass as bass
    import concourse.mybir as mybir
    import concourse.tile as tile
    from concourse import bacc

    f32 = mybir.dt.float32
    f16 = mybir.dt.float16
    dtp = _mydt(DT_P)
    dtg = _mydt(DT_GR)

    nc = bacc.Bacc("TRN2", target_bir_lowering=False, debug=False)

    hT = nc.dram_tensor("hT", (128, N), f16, kind="ExternalInput").ap()
    WlT = nc.dram_tensor("WlT", (128, 128), f16, kind="ExternalInput").ap()
    WrT = nc.dram_tensor("WrT", (128, 128), f16, kind="ExternalInput").ap()
    WS = nc.dram_tensor("WS", (128, BLK, 32), f16, kind="ExternalInput").ap()
    Wv = nc.dram_tensor("Wv", (128, 128), f16, kind="ExternalInput").ap()
    ident = nc.dram_tensor("ident", (128, 128), dtp, kind="ExternalInput").ap()
    out = nc.dram_tensor("out", (HALF, 128), f16, kind="ExternalOutput").ap()

    ADD = mybir.AluOpType.add
    MAX = mybir.AluOpType.max
    RELU = mybir.ActivationFunctionType.Relu
    EXP = mybir.ActivationFunctionType.Exp

    with tile.TileContext(nc) as tc:
        with (
            tc.tile_pool(name="singles", bufs=1) as singles,
            tc.tile_pool(name="xpool", bufs=XBUFS) as xpool,
            tc.tile_pool(name="ppool", bufs=PBUFS) as ppool,
            tc.tile_pool(name="ptpool", bufs=PBUFS) as ptpool,
            tc.tile_pool(name="small", bufs=6) as small,
            tc.tile_pool(name="ps_s", bufs=SBUFS, space=bass.MemorySpace.PSUM) as ps_s,
            tc.tile_pool(name="ps_t", bufs=2, space=bass.MemorySpace.PSUM) as ps_t,
            tc.tile_pool(name="ps_o", bufs=2, space=bass.MemorySpace.PSUM) as ps_o,
        ):
            # split loads across the two HWDGE queues (SP + ACT); operands of
            # the first projection land first
            sb_WlT = singles.tile([128, 128], f16)
            nc.sync.dma_start(sb_WlT, WlT)
            sb_WrT = singles.tile([128, 128], f16)
            nc.scalar.dma_start(sb_WrT, WrT)
            sb_hT = singles.tile([128, N], f16)
            nc.sync.dma_start(sb_hT, hT)
            sb_WS = singles.tile([128, BLK, 32], f16)
            nc.scalar.dma_start(sb_WS, WS)
            sb_Wv = singles.tile([128, 128], f16)
            nc.scalar.dma_start(sb_Wv, Wv)
            sb_id = singles.tile([128, 128], dtp)
            nc.scalar.dma_start(sb_id, ident)
            # all 16 blocks' normalized outputs land here; 16 tail DMAs
            # (2 half-ranges x 8 heads) extract the per-head diagonals
            stage16 = singles.tile([128, NBLK, 128], f16)

            # ---- prolog: projections (all-f16 matmuls, 1 cyc/col) ----
            # glT[(h,d), i_local] for this core's 256 rows (kept fp32: read
            # as per-partition scalars by the relu ops)
            g_ps = ps_s.tile([128, HALF], f32, tag="S")
            nc.tensor.matmul(g_ps, lhsT=sb_WlT, rhs=sb_hT[:, :HALF],
                             start=True, stop=True)
            sb_glT = singles.tile([128, HALF], f32)
            nc.vector.tensor_copy(sb_glT, g_ps)

            # grT[(h,d), j] for all 512 j
            r_ps = ps_s.tile([128, N], f32, tag="S")
            nc.tensor.matmul(r_ps, lhsT=sb_WrT, rhs=sb_hT, start=True, stop=True)
            sb_grT = singles.tile([128, N], dtg)
            nc.vector.tensor_copy(sb_grT, r_ps)

            # gr_cat[j, (h,d)] in 4 chunks of 128 j + trailing ones column
            # (the out-proj then also produces the softmax denominator Z)
            sb_grcat = singles.tile([128, 4, 129], dtg)
            nc.vector.memset(sb_grcat[:, :, 128], 1.0)
            for cch in range(4):
                c_ps = ps_o.tile([128, 128], f32, tag="O")
                nc.tensor.matmul(
                    c_ps,
                    lhsT=sb_hT[:, cch * 128 : (cch + 1) * 128],
                    rhs=sb_WrT,
                    start=True,
                    stop=True,
                )
                if cch % 2 == 0:
                    nc.scalar.copy(sb_grcat[:, cch, 0:128], c_ps)
                else:
                    nc.vector.tensor_copy(sb_grcat[:, cch, 0:128], c_ps)

            # ---- main loop: 16 blocks of 16 token-rows ---------------
            # Tail of block k is emitted inside block k+1's q loop so the
            # per-engine program order keeps every engine streaming:
            #  - tail1 (exp/transpose/PT-copies/out-proj) right after q0
            #  - tail2 (reciprocal on DVE + final relu on ACT) at q=TAIL2_Q,
            #    by which time the out-projection has surely retired, so the
            #    in-order DVE/ACT never stall on it.
            def engine_for_q(q):
                if q >= BLK - ACT_QS:
                    return "a"
                if q >= BLK - ACT_QS - POOL_QS:
                    return "p"
                return "d"

            def make_tails(blk, S_ps):
                P = ppool.tile([128, N], dtp, tag="P")
                T_ps = ps_t.tile([128, 4, 128], dtp, tag="T")
                PT = ptpool.tile([128, 4, 128], dtp, tag="PT")
                ow = 129
                O_ps = ps_o.tile([128, ow], f32, tag="O")

                def tail1():
                    nc.scalar.activation(P, S_ps, EXP)
                    for cch in range(4):
                        nc.tensor.transpose(
                            T_ps[:, cch], P[:, cch * 128 : (cch + 1) * 128],
                            sb_id,
                        )
                    for cch in range(4):
                        e = PT_ASSIGN[cch % len(PT_ASSIGN)]
                        if e == "p":
                            nc.gpsimd.tensor_copy(PT[:, cch], T_ps[:, cch])
                        elif e == "a":
                            nc.scalar.copy(PT[:, cch], T_ps[:, cch])
                        else:
                            nc.vector.tensor_copy(PT[:, cch], T_ps[:, cch])
                    for cch in range(4):
                        nc.tensor.matmul(
                            O_ps,
                            lhsT=PT[:, cch],
                            rhs=sb_grcat[:, cch],
                            start=(cch == 0),
                            stop=(cch == 3),
                        )

                Zi = small.tile([128, 1], f32, tag="Zi", name=f"Zi{blk}")

                def tail2a():
                    nc.vector.reciprocal(Zi, O_ps[:, 128:129])

                def tail2b():
                    nc.scalar.activation(
                        stage16[:, blk], O_ps[:, 0:128], RELU, bias=0.0,
                        scale=Zi,
                    )

                return tail1, tail2a, tail2b

            pending = []  # [(tail1, tail2)] of the previous block
            for blk in range(NBLK):
                S_ps = ps_s.tile([128, N], f32, tag="S", name=f"S_{blk}")
                for q in range(BLK):
                    i = blk * BLK + q
                    X = xpool.tile([128, N], f16, tag="X")
                    gl_col = sb_glT[:, i : i + 1]
                    e = engine_for_q(q)
                    if e == "p":
                        nc.gpsimd.tensor_scalar(X, sb_grT, gl_col, 0.0, ADD, MAX)
                    elif e == "a":
                        nc.scalar.activation(X, sb_grT, RELU, bias=gl_col,
                                             scale=1.0)
                    else:
                        nc.vector.tensor_scalar(X, sb_grT, gl_col, 0.0, ADD, MAX)
                    # S row for (i2=q, h) is 32*(q%4) + 8*(q//4) + h: four
                    # consecutive q's use the four 32-col PE array tiles and
                    # the in-tile column offset 8*(q//4) places later q's
                    # (weights are zero outside their 8 columns, so the
                    # start=True of q<4 resets the whole 32-row group once)
                    gq = q % 4
                    nc.tensor.matmul(
                        S_ps[32 * gq : 32 * gq + 32, :],
                        lhsT=sb_WS[:, q],
                        rhs=X,
                        start=(q < 4),
                        stop=False,
                        tile_position=(0, 32 * gq),
                        skip_group_check=True,
                    )
                    if q == 0 and pending:
                        pending[0][0]()
                    if q == TAIL2_Q and pending:
                        pending[0][1]()
                nc.tensor.matmul(
                    S_ps,
                    lhsT=sb_Wv,
                    rhs=sb_grT,
                    start=False,
                    stop=True,
                    skip_group_check=True,
                )
                if pending:
                    pending.pop(0)[2]()
                pending.append(make_tails(blk, S_ps))
            for t1, t2a, t2b in pending:
                t1()
                t2a()
                t2b()

            # diagonal extraction: out[blk*16+i2, h*16+d] = stage16[h*16+i2,
            # blk, h*16+d].  Two half-ranges per head so the first 8 blocks'
            # results stream out while the last 8 compute.
            for half in range(2):
                b0 = half * (NBLK // 2)
                for hh in range(N_HEADS):
                    src = stage16[
                        hh * 16 : (hh + 1) * 16, b0 : b0 + NBLK // 2,
                        hh * 16 : (hh + 1) * 16,
                    ]
                    dst = bass.AP(
                        tensor=out.tensor,
                        offset=b0 * BLK * 128 + hh * 16,
                        # dims iterate (i2, blk, d) matching src
                        ap=[[128, BLK], [BLK * 128, NBLK // 2], [1, 16]],
                    )
                    eng = nc.sync if hh % 2 == 0 else nc.scalar
                    eng.dma_start(dst, src)

    nc.compile()
    return nc


@functools.lru_cache(maxsize=1)
def get_program():
    return build_program()


def host_prep(h, Wl, Wr, Wak):
    """Build per-core input maps (all numpy, no device work)."""
    h = np.asarray(h, dtype=np.float32)
    Wl = np.asarray(Wl, dtype=np.float32)
    Wr = np.asarray(Wr, dtype=np.float32)
    Wak = np.asarray(Wak, dtype=np.float32)

    hT_all = np.ascontiguousarray(h.transpose(0, 2, 1))          # (B, 128, N)
    WlT = np.ascontiguousarray(
        Wl.transpose(1, 0, 2).reshape(INPUT_DIM, N_HEADS * HEAD_DIM)
    ).astype(np.float16)
    WrT = np.ascontiguousarray(
        Wr.transpose(1, 0, 2).reshape(INPUT_DIM, N_HEADS * HEAD_DIM)
    ).astype(np.float16)

    # WS[q][(h,d), 8*(q//4)+h] = 0.8 * Wak[h]; S row = 32*(q%4)+8*(q//4)+h
    WS = np.zeros((128, BLK, 32), dtype=np.float32)
    for hh in range(N_HEADS):
        for q in range(BLK):
            WS[hh * 16 : hh * 16 + 16, q, (q // 4) * 8 + hh] = 0.8 * Wak[hh]
    # Wv adds 0.2*v_hj to every S row (same scrambled row order)
    Wv = np.zeros((128, 128), dtype=np.float32)
    for hh in range(N_HEADS):
        for i2 in range(BLK):
            row = 32 * (i2 % 4) + 8 * (i2 // 4) + hh
            Wv[hh * 16 : hh * 16 + 16, row] = 0.2 * Wak[hh]
    WS = WS.astype(np.float16)
    Wv = Wv.astype(np.float16)
    # transpose permutation: row 32*(i2%4)+8*(i2//4)+h  ->  col h*16+i2
    ident = np.zeros((128, 128), dtype=np.float32)
    for hh in range(N_HEADS):
        for i2 in range(BLK):
            ident[32 * (i2 % 4) + 8 * (i2 // 4) + hh, hh * 16 + i2] = 1.0
    ident = ident.astype(_npdt(DT_P))

    in_maps = []
    for c in range(N_CORES):
        b = c // 2
        i0 = (c % 2) * HALF
        hT = hT_all[b]
        # j-permutation: own token half first (queries are cols 0:256)
        hTc = np.concatenate(
            [hT[:, i0 : i0 + HALF], hT[:, HALF - i0 : N - i0]], axis=1
        ).astype(np.float16)
        in_maps.append(
            {
                "hT": np.ascontiguousarray(hTc),
                "WlT": WlT,
                "WrT": WrT,
                "WS": WS,
                "Wv": Wv,
                "ident": ident,
            }
        )
    return in_maps


def run_on_cores(in_maps, trace=False):
    from concourse.bass_utils import run_bass_kernel_spmd

    nc = get_program()
    return run_bass_kernel_spmd(
        nc, in_maps, core_ids=list(range(N_CORES)), trace=trace
    )


def kernel(h, mask, Wl, Wr, Wak):
    """Full-input / full-output entry point (mask is all-False by problem
    construction; masked-off attention is a no-op and is not computed)."""
    in_maps = host_prep(h, Wl, Wr, Wak)
    res = run_on_cores(in_maps, trace=False)
    full = np.empty((B, N, INPUT_DIM), dtype=np.float32)
    for c in range(N_CORES):
        b = c // 2
        i0 = (c % 2) * HALF
        full[b, i0 : i0 + HALF] = res.results[c]["out"].astype(np.float32)
    return full


# revision 22
# speedup vs baseline: 3.2154x; 3.2154x over previous
"""GATv2 multi-head attention kernel for Trainium2 (8 NeuronCores).

Problem: nn_GATv2MHA  (b=4, n=512, input_dim=128, 8 heads x head_dim 16)
  g_l = einsum('bni,hid->hbnd', h, Wl); g_r likewise
  e = leaky_relu(g_l[:,:,:,None,:] + g_r[:,:,None,:,:], 0.2)
  scores = einsum('hbijd,hd->hbij', e, Wak);  attn = softmax(scores, -1)
  out = relu(einsum('hbij,hbjd->bihd', attn, g_r)).reshape(b, n, -1)

Sharding: data-parallel over (batch, token-half): core c handles batch c//2,
token rows [(c%2)*256, (c%2)*256+256).  No collectives.  The j (key) axis is
permuted per-core so the core's own half comes first — softmax over j is
permutation invariant, which lets one hT input serve as both the full key set
and the query slice (cols 0:256).

Math: LeakyRelu(x) = 0.2*x + 0.8*relu(x); the 0.2*u_i row-constant cancels in
softmax, so  scores'[row, j] = 0.8 * sum_d a_hd relu(gl_hid + gr_hjd)
                             + 0.2 * v_hj          (v_hj = sum_d a_hd gr_hjd)
Per i-row one tensor_scalar(add,max) makes X = relu(gl_col + grT) (split
DVE/ACT), and one PE matmul with the m=32 col-tiled weight WS_q contracts d
per head into S row 32*(q%4)+8*(q//4)+h (four consecutive q's use the four
32-col PE array tiles); the Wv matmul adds 0.2*v_j.  Softmax: no max
subtraction — scores are bounded (|s| < ~30) so P = exp(S) in bf16 cannot
overflow, and partial sums stay in f32 PSUM.  P is transposed by the DMA
XBAR (SBUF->SBUF, dispatched from the otherwise-idle SP engine), so the PE
never spends transpose cycles and no PSUM round-trip copy is needed.  A ones
column appended to gr_cat makes the out-projection emit the softmax
denominator Z as column 128; normalize+relu runs on DVE.  Output extraction
DMAs ride the gpsimd software DGE queue, pre-dispatched so the final
transfers fire the moment the last block's results land."""

import functools
import os

import numpy as np
import ml_dtypes

N_HEADS = 8
INPUT_DIM = 128
HEAD_DIM = 16
B = 4
N = 512
N_CORES = 8
HALF = N // 2          # token rows per core
BLK = 16               # i-rows per block (16 i x 8 heads = 128 S rows)
NBLK = HALF // BLK     # 16 blocks per core

# ---- tunables (overridable via env for experiments) ----------------------
ACT_QS = int(os.environ.get("GAT_ACT_QS", "5"))     # X-ops on ScalarE (last q)
XBUFS = int(os.environ.get("GAT_XBUFS", "12"))
SBUFS = int(os.environ.get("GAT_SBUFS", "6"))       # PSUM score banks
PBUFS = int(os.environ.get("GAT_PBUFS", "2"))
TAIL_O_Q = int(os.environ.get("GAT_TAIL_O_Q", "8"))   # out-proj emission q
TAIL_R_Q = int(os.environ.get("GAT_TAIL_R_Q", "12"))  # reciprocal emission q
PT_MODE = os.environ.get("GAT_PT_MODE", "xbar")     # xbar | eng


def build_program():
    """Build + compile the (identical-across-cores) Bass program."""
    import concourse.bass as bass
    import concourse.mybir as mybir
    import concourse.tile as tile
    from concourse import bacc

    f32 = mybir.dt.float32
    f16 = mybir.dt.float16
    bf16 = mybir.dt.bfloat16

    nc = bacc.Bacc("TRN2", target_bir_lowering=False, debug=False)

    hT = nc.dram_tensor("hT", (128, N), f16, kind="ExternalInput").ap()
    WlT = nc.dram_tensor("WlT", (128, 128), f16, kind="ExternalInput").ap()
    WrT = nc.dram_tensor("WrT", (128, 128), f16, kind="ExternalInput").ap()
    WS = nc.dram_tensor("WS", (128, BLK, 32), f16, kind="ExternalInput").ap()
    Wv = nc.dram_tensor("Wv", (128, 128), f16, kind="ExternalInput").ap()
    # raw staging layout: row = scrambled S row, cols = (blk, h*16+d); the
    # host-side unshard gathers the per-head diagonals out of it
    out = nc.dram_tensor("out", (128, NBLK * 128), f16,
                         kind="ExternalOutput").ap()

    ADD = mybir.AluOpType.add
    MAX = mybir.AluOpType.max
    MULT = mybir.AluOpType.mult
    RELU = mybir.ActivationFunctionType.Relu
    EXP = mybir.ActivationFunctionType.Exp

    with tile.TileContext(nc) as tc:
        with (
            tc.tile_pool(name="singles", bufs=1) as singles,
            tc.tile_pool(name="xpool", bufs=XBUFS) as xpool,
            tc.tile_pool(name="ppool", bufs=PBUFS) as ppool,
            tc.tile_pool(name="ptpool", bufs=PBUFS) as ptpool,
            tc.tile_pool(name="small", bufs=6) as small,
            tc.tile_pool(name="ps_s", bufs=SBUFS, space=bass.MemorySpace.PSUM) as ps_s,
            tc.tile_pool(name="ps_t", bufs=2, space=bass.MemorySpace.PSUM) as ps_t,
            tc.tile_pool(name="ps_o", bufs=2, space=bass.MemorySpace.PSUM) as ps_o,
        ):
            # input loads split across the two HWDGE queues (SP + ACT)
            sb_WlT = singles.tile([128, 128], f16)
            nc.sync.dma_start(sb_WlT, WlT)
            sb_WrT = singles.tile([128, 128], f16)
            nc.scalar.dma_start(sb_WrT, WrT)
            sb_hT = singles.tile([128, N], f16)
            nc.sync.dma_start(sb_hT, hT)
            sb_WS = singles.tile([128, BLK, 32], f16)
            nc.scalar.dma_start(sb_WS, WS)
            sb_Wv = singles.tile([128, 128], f16)
            nc.scalar.dma_start(sb_Wv, Wv)
            # all 16 blocks' normalized outputs land here; 16 pre-dispatched
            # DMAs (2 half-ranges x 8 heads) extract the per-head diagonals
            stage16 = singles.tile([128, NBLK, 128], f16)

            # ---- prolog: projections (all-f16 matmuls, 1 cyc/col) ----
            g_ps = ps_s.tile([128, HALF], f32, tag="S")
            nc.tensor.matmul(g_ps, lhsT=sb_WlT, rhs=sb_hT[:, :HALF],
                             start=True, stop=True)
            sb_glT = singles.tile([128, HALF], f32)
            nc.vector.tensor_copy(sb_glT, g_ps)

            r_ps = ps_s.tile([128, N], f32, tag="S")
            nc.tensor.matmul(r_ps, lhsT=sb_WrT, rhs=sb_hT, start=True, stop=True)
            sb_grT = singles.tile([128, N], f16)
            nc.vector.tensor_copy(sb_grT, r_ps)

            # gr_cat[j, (h,d)] via XBAR transpose + ones column at 128;
            # chunks are padded to 256 so every transpose lands on a
            # 512B-aligned destination (the XBAR miscomputes otherwise)
            sb_grcat = singles.tile([128, 4, 256], f16)
            nc.vector.memset(sb_grcat[:, :, 128], 1.0)
            for cch in range(4):
                eng = nc.sync if cch % 2 == 0 else nc.scalar
                eng.dma_start_transpose(
                    sb_grcat[:, cch, 0:128],
                    sb_grT[:, cch * 128 : (cch + 1) * 128],
                )

            # ---- main loop: 16 blocks of 16 token-rows ---------------
            # Tail pieces of block k are emitted inside block k+1's q loop at
            # staggered points so every in-order engine keeps streaming:
            #   q0:        exp (ACT) + 4 XBAR transpose dispatches (SP)
            #   TAIL_O_Q:  out-projection (PE reaches it ~1.8us in, after the
            #              XBAR transfers have surely landed)
            #   TAIL_R_Q:  reciprocal (DVE program reaches it even later)
            #   post-Wv:   normalize+relu (DVE)
            def make_tails(blk, S_ps):
                P = ppool.tile([128, N], bf16, tag="P")
                PT = ptpool.tile([128, 4, 128], bf16, tag="PT")
                T_ps = (
                    ps_t.tile([128, 4, 128], bf16, tag="T")
                    if PT_MODE == "eng" else None
                )
                O_ps = ps_o.tile([128, 129], f32, tag="O")
                Zi = small.tile([128, 1], f32, tag="Zi", name=f"Zi{blk}")

                def tail1a():
                    nc.scalar.activation(P, S_ps, EXP)
                    if PT_MODE == "xbar":
                        for cch in range(4):
                            nc.sync.dma_start_transpose(
                                PT[:, cch], P[:, cch * 128 : (cch + 1) * 128]
                            )

                def tail1b():
                    # O rows keep the scrambled S order 32*(i2%4)+8*(i2//4)+h;
                    # the host-side unshard gather resolves it for free
                    for cch in range(4):
                        nc.tensor.matmul(
                            O_ps,
                            lhsT=PT[:, cch],
                            rhs=sb_grcat[:, cch, 0:129],
                            start=(cch == 0),
                            stop=(cch == 3),
                        )

                def tail2a():
                    nc.vector.reciprocal(Zi, O_ps[:, 128:129])

                def tail2b():
                    nc.vector.tensor_scalar(
                        stage16[:, blk], O_ps[:, 0:128], Zi, 0.0, MULT, MAX
                    )

                def tail_eng_pt():
                    # fallback: PE transpose + engine copies (PT_MODE=eng)
                    for cch in range(4):
                        nc.tensor.transpose(
                            T_ps[:, cch], P[:, cch * 128 : (cch + 1) * 128],
                            sb_id,
                        )
                    for cch in range(4):
                        if cch % 2 == 0:
                            nc.vector.tensor_copy(PT[:, cch], T_ps[:, cch])
                        else:
                            nc.scalar.copy(PT[:, cch], T_ps[:, cch])

                if PT_MODE == "eng":
                    def tail1a_eng():
                        nc.scalar.activation(P, S_ps, EXP)
                        tail_eng_pt()
                    return tail1a_eng, tail1b, tail2a, tail2b
                return tail1a, tail1b, tail2a, tail2b

            if PT_MODE == "eng":
                ident = nc.dram_tensor(
                    "ident", (128, 128), bf16, kind="ExternalInput"
                ).ap()
                sb_id = singles.tile([128, 128], bf16)
                nc.scalar.dma_start(sb_id, ident)

            pending = []  # tail tuple of the previous block
            for blk in range(NBLK):
                S_ps = ps_s.tile([128, N], f32, tag="S", name=f"S_{blk}")
                for q in range(BLK):
                    i = blk * BLK + q
                    X = xpool.tile([128, N], f16, tag="X")
                    gl_col = sb_glT[:, i : i + 1]
                    if q >= BLK - ACT_QS:
                        nc.scalar.activation(X, sb_grT, RELU, bias=gl_col,
                                             scale=1.0)
                    else:
                        nc.vector.tensor_scalar(X, sb_grT, gl_col, 0.0, ADD, MAX)
                    # S row for (i2=q, h) is 32*(q%4) + 8*(q//4) + h: four
                    # consecutive q's use the four 32-col PE array tiles
                    # (weights are zero outside their 8 columns, so q<4's
                    # start=True resets each 32-row group exactly once)
                    gq = q % 4
                    nc.tensor.matmul(
                        S_ps[32 * gq : 32 * gq + 32, :],
                        lhsT=sb_WS[:, q],
                        rhs=X,
                        start=(q < 4),
                        stop=False,
                        tile_position=(0, 32 * gq),
                        skip_group_check=True,
                    )
                    if q == 0 and pending:
                        pending[0][0]()
                    if q == TAIL_O_Q and pending:
                        pending[0][1]()
                    if q == TAIL_R_Q and pending:
                        pending[0][2]()
                nc.tensor.matmul(
                    S_ps,
                    lhsT=sb_Wv,
                    rhs=sb_grT,
                    start=False,
                    stop=True,
                    skip_group_check=True,
                )
                if pending:
                    pending.pop(0)[3]()
                if blk == NBLK // 2 + 1:
                    # first half of the staging tile is complete: stream it
                    # out while the second half computes
                    nc.sync.dma_start(
                        out[:, : (NBLK // 2) * 128],
                        stage16[:, : NBLK // 2, :],
                    )
                pending.append(make_tails(blk, S_ps))
            for t1a, t1b, t2a, t2b in pending:
                t1a(); t1b(); t2a(); t2b()
            nc.scalar.dma_start(
                out[:, (NBLK // 2) * 128 :], stage16[:, NBLK // 2 :, :]
            )

    nc.compile()
    return nc


@functools.lru_cache(maxsize=1)
def get_program():
    return build_program()


def host_prep(h, Wl, Wr, Wak):
    """Build per-core input maps (all numpy, no device work)."""
    h = np.asarray(h, dtype=np.float32)
    Wl = np.asarray(Wl, dtype=np.float32)
    Wr = np.asarray(Wr, dtype=np.float32)
    Wak = np.asarray(Wak, dtype=np.float32)

    hT_all = np.ascontiguousarray(h.transpose(0, 2, 1))          # (B, 128, N)
    WlT = np.ascontiguousarray(
        Wl.transpose(1, 0, 2).reshape(INPUT_DIM, N_HEADS * HEAD_DIM)
    ).astype(np.float16)
    WrT = np.ascontiguousarray(
        Wr.transpose(1, 0, 2).reshape(INPUT_DIM, N_HEADS * HEAD_DIM)
    ).astype(np.float16)

    # WS[q][(h,d), 8*(q//4)+h] = 0.8 * Wak[h]; S row = 32*(q%4)+8*(q//4)+h
    WS = np.zeros((128, BLK, 32), dtype=np.float32)
    for hh in range(N_HEADS):
        for q in range(BLK):
            WS[hh * 16 : hh * 16 + 16, q, (q // 4) * 8 + hh] = 0.8 * Wak[hh]
    # Wv adds 0.2*v_hj to every S row (same scrambled row order)
    Wv = np.zeros((128, 128), dtype=np.float32)
    for hh in range(N_HEADS):
        for i2 in range(BLK):
            row = 32 * (i2 % 4) + 8 * (i2 // 4) + hh
            Wv[hh * 16 : hh * 16 + 16, row] = 0.2 * Wak[hh]
    WS = WS.astype(np.float16)
    Wv = Wv.astype(np.float16)

    in_maps = []
    for c in range(N_CORES):
        b = c // 2
        i0 = (c % 2) * HALF
        hT = hT_all[b]
        # j-permutation: own token half first (queries are cols 0:256)
        hTc = np.concatenate(
            [hT[:, i0 : i0 + HALF], hT[:, HALF - i0 : N - i0]], axis=1
        ).astype(np.float16)
        m = {
            "hT": np.ascontiguousarray(hTc),
            "WlT": WlT,
            "WrT": WrT,
            "WS": WS,
            "Wv": Wv,
        }
        if PT_MODE == "eng":
            # transpose permutation: row 32*(i2%4)+8*(i2//4)+h -> col h*16+i2
            ident = np.zeros((128, 128), dtype=np.float32)
            for hh in range(N_HEADS):
                for i2 in range(BLK):
                    ident[32 * (i2 % 4) + 8 * (i2 // 4) + hh,
                          hh * 16 + i2] = 1.0
            m["ident"] = ident.astype(ml_dtypes.bfloat16)
        in_maps.append(m)
    return in_maps


def run_on_cores(in_maps, trace=False):
    from concourse.bass_utils import run_bass_kernel_spmd

    nc = get_program()
    return run_bass_kernel_spmd(
        nc, in_maps, core_ids=list(range(N_CORES)), trace=trace
    )


_I2 = np.arange(BLK)
_ROWS = 32 * (_I2 % 4)[:, None] + 8 * (_I2 // 4)[:, None] + np.arange(8)[None, :]


def _unshard(raw):
    """Gather the per-head diagonals out of the raw scrambled staging dump:
    out[blk*16+i2, h*16+d] = raw[32*(i2%4)+8*(i2//4)+h, blk, h*16+d]."""
    stage = np.asarray(raw, dtype=np.float32).reshape(128, NBLK, N_HEADS, 16)
    sel = stage[_ROWS]                        # [i2, h, blk, h2, d]
    hh = np.arange(N_HEADS)
    picked = sel[_I2[:, None], hh[None, :], :, hh[None, :], :]  # [i2, h, blk, d]
    return picked.transpose(2, 0, 1, 3).reshape(HALF, 128)


def kernel(h, mask, Wl, Wr, Wak):
    """Full-input / full-output entry point (mask is all-False by problem
    construction; masked-off attention is a no-op and is not computed)."""
    in_maps = host_prep(h, Wl, Wr, Wak)
    res = run_on_cores(in_maps, trace=False)
    full = np.empty((B, N, INPUT_DIM), dtype=np.float32)
    for c in range(N_CORES):
        b = c // 2
        i0 = (c % 2) * HALF
        full[b, i0 : i0 + HALF] = _unshard(res.results[c]["out"])
    return full


# revision 25
# speedup vs baseline: 5.2456x; 1.6314x over previous
"""GATv2 multi-head attention kernel for Trainium2 (8 NeuronCores).

Problem: nn_GATv2MHA  (b=4, n=512, input_dim=128, 8 heads x head_dim 16)
  g_l = einsum('bni,hid->hbnd', h, Wl); g_r likewise
  e = leaky_relu(g_l[:,:,:,None,:] + g_r[:,:,None,:,:], 0.2)
  scores = einsum('hbijd,hd->hbij', e, Wak);  attn = softmax(scores, -1)
  out = relu(einsum('hbij,hbjd->bihd', attn, g_r)).reshape(b, n, -1)

Sharding: data-parallel over (batch, token-half): core c handles batch c//2,
token rows [(c%2)*256, (c%2)*256+256).  No collectives.  The j (key) axis is
permuted per-core so the core's own half comes first — softmax over j is
permutation invariant, which lets one hT input serve as both the full key set
and the query slice (cols 0:256).

Math: LeakyRelu(x) = 0.2*x + 0.8*relu(x); the 0.2*u_i row-constant cancels in
softmax, so  scores'[row, j] = 0.8 * sum_d a_hd relu(gl_hid + gr_hjd)
                             + 0.2 * v_hj          (v_hj = sum_d a_hd gr_hjd)
Per i-row one tensor_scalar(add,max) makes X = relu(gl_col + grT) (split
DVE/ACT), and one PE matmul with the m=32 col-tiled weight WS_q contracts d
per head into S row 32*(q%4)+8*(q//4)+h (four consecutive q's use the four
32-col PE array tiles); the Wv matmul adds 0.2*v_j.  Softmax: no max
subtraction — scores are bounded (|s| < ~30) so P = exp(S) in bf16 cannot
overflow, and partial sums stay in f32 PSUM.  P is transposed by the DMA
XBAR (SBUF->SBUF, dispatched from the otherwise-idle SP engine), so the PE
never spends transpose cycles and no PSUM round-trip copy is needed.  A ones
column appended to gr_cat makes the out-projection emit the softmax
denominator Z as column 128; normalize+relu runs on DVE.  Output extraction
DMAs ride the gpsimd software DGE queue, pre-dispatched so the final
transfers fire the moment the last block's results land."""

import functools
import os

import numpy as np
import ml_dtypes

N_HEADS = 8
INPUT_DIM = 128
HEAD_DIM = 16
B = 4
N = 512
N_CORES = 8
HALF = N // 2          # token rows per core
BLK = 16               # i-rows per block (16 i x 8 heads = 128 S rows)
NBLK = HALF // BLK     # 16 blocks per core

# ---- tunables (overridable via env for experiments) ----------------------
ACT_QS = int(os.environ.get("GAT_ACT_QS", "5"))     # X-ops on ScalarE (last q)
XBUFS = int(os.environ.get("GAT_XBUFS", "12"))
SBUFS = int(os.environ.get("GAT_SBUFS", "6"))       # PSUM score banks
PBUFS = int(os.environ.get("GAT_PBUFS", "2"))
TAIL_O_Q = int(os.environ.get("GAT_TAIL_O_Q", "10"))  # out-proj emission q
TAIL_R_Q = int(os.environ.get("GAT_TAIL_R_Q", "13"))  # reciprocal emission q
PT_MODE = os.environ.get("GAT_PT_MODE", "xbar")     # xbar | eng


def build_program():
    """Build + compile the (identical-across-cores) Bass program."""
    import concourse.bass as bass
    import concourse.mybir as mybir
    import concourse.tile as tile
    from concourse import bacc

    f32 = mybir.dt.float32
    f16 = mybir.dt.float16
    bf16 = mybir.dt.bfloat16

    nc = bacc.Bacc("TRN2", target_bir_lowering=False, debug=False)

    hT = nc.dram_tensor("hT", (128, N), f16, kind="ExternalInput").ap()
    WlT = nc.dram_tensor("WlT", (128, 128), f16, kind="ExternalInput").ap()
    WrT = nc.dram_tensor("WrT", (128, 128), f16, kind="ExternalInput").ap()
    WS = nc.dram_tensor("WS", (128, BLK, 32), f16, kind="ExternalInput").ap()
    Wv = nc.dram_tensor("Wv", (128, 128), f16, kind="ExternalInput").ap()
    # raw staging layout: row = scrambled S row, cols = (blk, h*16+d); the
    # host-side unshard gathers the per-head diagonals out of it
    out = nc.dram_tensor("out", (128, NBLK * 128), f16,
                         kind="ExternalOutput").ap()

    ADD = mybir.AluOpType.add
    MAX = mybir.AluOpType.max
    MULT = mybir.AluOpType.mult
    RELU = mybir.ActivationFunctionType.Relu
    EXP = mybir.ActivationFunctionType.Exp

    with tile.TileContext(nc) as tc:
        with (
            tc.tile_pool(name="singles", bufs=1) as singles,
            tc.tile_pool(name="xpool", bufs=XBUFS) as xpool,
            tc.tile_pool(name="ppool", bufs=PBUFS) as ppool,
            tc.tile_pool(name="ptpool", bufs=PBUFS) as ptpool,
            tc.tile_pool(name="small", bufs=6) as small,
            tc.tile_pool(name="ps_s", bufs=SBUFS, space=bass.MemorySpace.PSUM) as ps_s,
            tc.tile_pool(name="ps_t", bufs=2, space=bass.MemorySpace.PSUM) as ps_t,
            tc.tile_pool(name="ps_o", bufs=2, space=bass.MemorySpace.PSUM) as ps_o,
        ):
            # input loads split across the two HWDGE queues (SP + ACT)
            sb_WlT = singles.tile([128, 128], f16)
            nc.sync.dma_start(sb_WlT, WlT)
            sb_WrT = singles.tile([128, 128], f16)
            nc.scalar.dma_start(sb_WrT, WrT)
            sb_hT = singles.tile([128, N], f16)
            nc.sync.dma_start(sb_hT, hT)
            sb_WS = singles.tile([128, BLK, 32], f16)
            nc.scalar.dma_start(sb_WS, WS)
            sb_Wv = singles.tile([128, 128], f16)
            nc.scalar.dma_start(sb_Wv, Wv)
            # all 16 blocks' normalized outputs land here; 16 pre-dispatched
            # DMAs (2 half-ranges x 8 heads) extract the per-head diagonals
            stage16 = singles.tile([128, NBLK, 128], f16)

            # ---- prolog: projections (all-f16 matmuls, 1 cyc/col) ----
            g_ps = ps_s.tile([128, HALF], f32, tag="S")
            nc.tensor.matmul(g_ps, lhsT=sb_WlT, rhs=sb_hT[:, :HALF],
                             start=True, stop=True)
            sb_glT = singles.tile([128, HALF], f32)
            nc.vector.tensor_copy(sb_glT, g_ps)

            r_ps = ps_s.tile([128, N], f32, tag="S")
            nc.tensor.matmul(r_ps, lhsT=sb_WrT, rhs=sb_hT, start=True, stop=True)
            sb_grT = singles.tile([128, N], f16)
            nc.vector.tensor_copy(sb_grT, r_ps)

            # gr_cat[j, (h,d)] via XBAR transpose + ones column at 128;
            # chunks are padded to 256 so every transpose lands on a
            # 512B-aligned destination (the XBAR miscomputes otherwise)
            sb_grcat = singles.tile([128, 4, 256], f16)
            nc.vector.memset(sb_grcat[:, :, 128], 1.0)
            nc.sync.dma_start_transpose(sb_grcat[:, :, 0:128], sb_grT)

            # ---- main loop: 16 blocks of 16 token-rows ---------------
            # Tail pieces of block k are emitted inside block k+1's q loop at
            # staggered points so every in-order engine keeps streaming:
            #   q0:        exp (ACT) + 4 XBAR transpose dispatches (SP)
            #   TAIL_O_Q:  out-projection (PE reaches it ~1.8us in, after the
            #              XBAR transfers have surely landed)
            #   TAIL_R_Q:  reciprocal (DVE program reaches it even later)
            #   post-Wv:   normalize+relu (DVE)
            def make_tails(blk, S_ps):
                P = ppool.tile([128, N], bf16, tag="P")
                PT = ptpool.tile([128, 4, 128], bf16, tag="PT")
                T_ps = (
                    ps_t.tile([128, 4, 128], bf16, tag="T")
                    if PT_MODE == "eng" else None
                )
                O_ps = ps_o.tile([128, 129], f32, tag="O")
                Zi = small.tile([128, 1], f32, tag="Zi", name=f"Zi{blk}")

                def tail1a():
                    nc.scalar.activation(P, S_ps, EXP)
                    if PT_MODE == "xbar":
                        # one whole-tensor XBAR transpose: lands exactly in
                        # the chunked layout PT[:, c] = P[:, c*128:...].T
                        nc.sync.dma_start_transpose(PT, P)

                def tail1b():
                    # O rows keep the scrambled S order 32*(i2%4)+8*(i2//4)+h;
                    # the host-side unshard gather resolves it for free
                    for cch in range(4):
                        nc.tensor.matmul(
                            O_ps,
                            lhsT=PT[:, cch],
                            rhs=sb_grcat[:, cch, 0:129],
                            start=(cch == 0),
                            stop=(cch == 3),
                        )

                def tail2a():
                    nc.vector.reciprocal(Zi, O_ps[:, 128:129])

                def tail2b():
                    nc.vector.tensor_scalar(
                        stage16[:, blk], O_ps[:, 0:128], Zi, 0.0, MULT, MAX
                    )

                def tail_eng_pt():
                    # fallback: PE transpose + engine copies (PT_MODE=eng)
                    for cch in range(4):
                        nc.tensor.transpose(
                            T_ps[:, cch], P[:, cch * 128 : (cch + 1) * 128],
                            sb_id,
                        )
                    for cch in range(4):
                        if cch % 2 == 0:
                            nc.vector.tensor_copy(PT[:, cch], T_ps[:, cch])
                        else:
                            nc.scalar.copy(PT[:, cch], T_ps[:, cch])

                if PT_MODE == "eng":
                    def tail1a_eng():
                        nc.scalar.activation(P, S_ps, EXP)
                        tail_eng_pt()
                    return tail1a_eng, tail1b, tail2a, tail2b
                return tail1a, tail1b, tail2a, tail2b

            if PT_MODE == "eng":
                ident = nc.dram_tensor(
                    "ident", (128, 128), bf16, kind="ExternalInput"
                ).ap()
                sb_id = singles.tile([128, 128], bf16)
                nc.scalar.dma_start(sb_id, ident)

            pending = []  # tail tuple of the previous block
            for blk in range(NBLK):
                S_ps = ps_s.tile([128, N], f32, tag="S", name=f"S_{blk}")
                for q in range(BLK):
                    i = blk * BLK + q
                    X = xpool.tile([128, N], f16, tag="X")
                    gl_col = sb_glT[:, i : i + 1]
                    if q >= BLK - ACT_QS:
                        nc.scalar.activation(X, sb_grT, RELU, bias=gl_col,
                                             scale=1.0)
                    else:
                        nc.vector.tensor_scalar(X, sb_grT, gl_col, 0.0, ADD, MAX)
                    # S row for (i2=q, h) is 32*(q%4) + 8*(q//4) + h: four
                    # consecutive q's use the four 32-col PE array tiles
                    # (weights are zero outside their 8 columns, so q<4's
                    # start=True resets each 32-row group exactly once)
                    gq = q % 4
                    nc.tensor.matmul(
                        S_ps[32 * gq : 32 * gq + 32, :],
                        lhsT=sb_WS[:, q],
                        rhs=X,
                        start=(q < 4),
                        stop=False,
                        tile_position=(0, 32 * gq),
                        skip_group_check=True,
                    )
                    if q == 0 and pending:
                        pending[0][0]()
                    if q == TAIL_O_Q and pending:
                        pending[0][1]()
                    if q == TAIL_R_Q and pending:
                        pending[0][2]()
                nc.tensor.matmul(
                    S_ps,
                    lhsT=sb_Wv,
                    rhs=sb_grT,
                    start=False,
                    stop=True,
                    skip_group_check=True,
                )
                if pending:
                    pending.pop(0)[3]()
                if blk == NBLK // 2 + 1:
                    # first half of the staging tile is complete: stream it
                    # out while the second half computes
                    nc.sync.dma_start(
                        out[:, : (NBLK // 2) * 128],
                        stage16[:, : NBLK // 2, :],
                    )
                pending.append(make_tails(blk, S_ps))
            for t1a, t1b, t2a, t2b in pending:
                t1a(); t1b(); t2a(); t2b()
            nc.scalar.dma_start(
                out[:, (NBLK // 2) * 128 :], stage16[:, NBLK // 2 :, :]
            )

    nc.compile()
    return nc


@functools.lru_cache(maxsize=1)
def get_program():
    return build_program()


def host_prep(h, Wl, Wr, Wak):
    """Build per-core input maps (all numpy, no device work)."""
    h = np.asarray(h, dtype=np.float32)
    Wl = np.asarray(Wl, dtype=np.float32)
    Wr = np.asarray(Wr, dtype=np.float32)
    Wak = np.asarray(Wak, dtype=np.float32)

    hT_all = np.ascontiguousarray(h.transpose(0, 2, 1))          # (B, 128, N)
    WlT = np.ascontiguousarray(
        Wl.transpose(1, 0, 2).reshape(INPUT_DIM, N_HEADS * HEAD_DIM)
    ).astype(np.float16)
    WrT = np.ascontiguousarray(
        Wr.transpose(1, 0, 2).reshape(INPUT_DIM, N_HEADS * HEAD_DIM)
    ).astype(np.float16)

    # WS[q][(h,d), 8*(q//4)+h] = 0.8 * Wak[h]; S row = 32*(q%4)+8*(q//4)+h
    WS = np.zeros((128, BLK, 32), dtype=np.float32)
    for hh in range(N_HEADS):
        for q in range(BLK):
            WS[hh * 16 : hh * 16 + 16, q, (q // 4) * 8 + hh] = 0.8 * Wak[hh]
    # Wv adds 0.2*v_hj to every S row (same scrambled row order)
    Wv = np.zeros((128, 128), dtype=np.float32)
    for hh in range(N_HEADS):
        for i2 in range(BLK):
            row = 32 * (i2 % 4) + 8 * (i2 // 4) + hh
            Wv[hh * 16 : hh * 16 + 16, row] = 0.2 * Wak[hh]
    WS = WS.astype(np.float16)
    Wv = Wv.astype(np.float16)

    in_maps = []
    for c in range(N_CORES):
        b = c // 2
        i0 = (c % 2) * HALF
        hT = hT_all[b]
        # j-permutation: own token half first (queries are cols 0:256)
        hTc = np.concatenate(
            [hT[:, i0 : i0 + HALF], hT[:, HALF - i0 : N - i0]], axis=1
        ).astype(np.float16)
        m = {
            "hT": np.ascontiguousarray(hTc),
            "WlT": WlT,
            "WrT": WrT,
            "WS": WS,
            "Wv": Wv,
        }
        if PT_MODE == "eng":
            # transpose permutation: row 32*(i2%4)+8*(i2//4)+h -> col h*16+i2
            ident = np.zeros((128, 128), dtype=np.float32)
            for hh in range(N_HEADS):
                for i2 in range(BLK):
                    ident[32 * (i2 % 4) + 8 * (i2 // 4) + hh,
                          hh * 16 + i2] = 1.0
            m["ident"] = ident.astype(ml_dtypes.bfloat16)
        in_maps.append(m)
    return in_maps


def run_on_cores(in_maps, trace=False):
    from concourse.bass_utils import run_bass_kernel_spmd

    nc = get_program()
    return run_bass_kernel_spmd(
        nc, in_maps, core_ids=list(range(N_CORES)), trace=trace
    )


_I2 = np.arange(BLK)
_ROWS = 32 * (_I2 % 4)[:, None] + 8 * (_I2 // 4)[:, None] + np.arange(8)[None, :]


def _unshard(raw):
    """Gather the per-head diagonals out of the raw scrambled staging dump:
    out[blk*16+i2, h*16+d] = raw[32*(i2%4)+8*(i2//4)+h, blk, h*16+d]."""
    stage = np.asarray(raw, dtype=np.float32).reshape(128, NBLK, N_HEADS, 16)
    sel = stage[_ROWS]                        # [i2, h, blk, h2, d]
    hh = np.arange(N_HEADS)
    picked = sel[_I2[:, None], hh[None, :], :, hh[None, :], :]  # [i2, h, blk, d]
    return picked.transpose(2, 0, 1, 3).reshape(HALF, 128)


def kernel(h, mask, Wl, Wr, Wak):
    """Full-input / full-output entry point (mask is all-False by problem
    construction; masked-off attention is a no-op and is not computed)."""
    in_maps = host_prep(h, Wl, Wr, Wak)
    res = run_on_cores(in_maps, trace=False)
    full = np.empty((B, N, INPUT_DIM), dtype=np.float32)
    for c in range(N_CORES):
        b = c // 2
        i0 = (c % 2) * HALF
        full[b, i0 : i0 + HALF] = _unshard(res.results[c]["out"])
    return full


# revision 27
# speedup vs baseline: 5.6973x; 1.0861x over previous
"""GATv2 multi-head attention kernel for Trainium2 (8 NeuronCores).

Problem: nn_GATv2MHA  (b=4, n=512, input_dim=128, 8 heads x head_dim 16)
  g_l = einsum('bni,hid->hbnd', h, Wl); g_r likewise
  e = leaky_relu(g_l[:,:,:,None,:] + g_r[:,:,None,:,:], 0.2)
  scores = einsum('hbijd,hd->hbij', e, Wak);  attn = softmax(scores, -1)
  out = relu(einsum('hbij,hbjd->bihd', attn, g_r)).reshape(b, n, -1)

Sharding: data-parallel over (batch, token-half): core c handles batch c//2,
token rows [(c%2)*256, (c%2)*256+256).  No collectives.  The j (key) axis is
permuted per-core so the core's own half comes first — softmax over j is
permutation invariant, which lets one hT input serve as both the full key set
and the query slice (cols 0:256).

Math: LeakyRelu(x) = 0.2*x + 0.8*relu(x); the 0.2*u_i row-constant cancels in
softmax, so  scores'[row, j] = 0.8 * sum_d a_hd relu(gl_hid + gr_hjd)
                             + 0.2 * v_hj          (v_hj = sum_d a_hd gr_hjd)
Per i-row one tensor_scalar(add,max) makes X = relu(gl_col + grT) (split
DVE/ACT), and one PE matmul with the m=32 col-tiled weight WS_q contracts d
per head into S row 32*(q%4)+8*(q//4)+h (four consecutive q's use the four
32-col PE array tiles); the Wv matmul adds 0.2*v_j.  Softmax: no max
subtraction — scores are bounded (|s| < ~30) so P = exp(S) in bf16 cannot
overflow, and partial sums stay in f32 PSUM.  P is transposed by the DMA
XBAR (SBUF->SBUF, dispatched from the otherwise-idle SP engine), so the PE
never spends transpose cycles and no PSUM round-trip copy is needed.  A ones
column appended to gr_cat makes the out-projection emit the softmax
denominator Z as column 128; normalize+relu runs on DVE.  Output extraction
DMAs ride the gpsimd software DGE queue, pre-dispatched so the final
transfers fire the moment the last block's results land."""

import functools
import os

import numpy as np
import ml_dtypes

N_HEADS = 8
INPUT_DIM = 128
HEAD_DIM = 16
B = 4
N = 512
N_CORES = 8
HALF = N // 2          # token rows per core
BLK = 16               # i-rows per block (16 i x 8 heads = 128 S rows)
NBLK = HALF // BLK     # 16 blocks per core

# ---- tunables (overridable via env for experiments) ----------------------
ACT_QS = int(os.environ.get("GAT_ACT_QS", "5"))     # X-ops on ScalarE (last q)
XBUFS = int(os.environ.get("GAT_XBUFS", "12"))
SBUFS = int(os.environ.get("GAT_SBUFS", "5"))       # PSUM score banks
PBUFS = int(os.environ.get("GAT_PBUFS", "2"))
OBUFS = int(os.environ.get("GAT_OBUFS", "3"))
TAIL_O_Q = int(os.environ.get("GAT_TAIL_O_Q", "10"))  # out-proj emission q
TAIL_R_Q = int(os.environ.get("GAT_TAIL_R_Q", "13"))  # reciprocal emission q
PT_MODE = os.environ.get("GAT_PT_MODE", "xbar")     # xbar | eng


def build_program():
    """Build + compile the (identical-across-cores) Bass program."""
    import concourse.bass as bass
    import concourse.mybir as mybir
    import concourse.tile as tile
    from concourse import bacc

    f32 = mybir.dt.float32
    f16 = mybir.dt.float16
    bf16 = mybir.dt.bfloat16

    nc = bacc.Bacc("TRN2", target_bir_lowering=False, debug=False)

    hT = nc.dram_tensor("hT", (128, N), f16, kind="ExternalInput").ap()
    WlT = nc.dram_tensor("WlT", (128, 128), f16, kind="ExternalInput").ap()
    WrT = nc.dram_tensor("WrT", (128, 128), f16, kind="ExternalInput").ap()
    WS = nc.dram_tensor("WS", (128, BLK, 32), f16, kind="ExternalInput").ap()
    Wv = nc.dram_tensor("Wv", (128, 128), f16, kind="ExternalInput").ap()
    # raw staging layout: row = scrambled S row, cols = (blk, h*16+d); the
    # host-side unshard gathers the per-head diagonals out of it
    out = nc.dram_tensor("out", (128, NBLK * 128), f16,
                         kind="ExternalOutput").ap()

    ADD = mybir.AluOpType.add
    MAX = mybir.AluOpType.max
    MULT = mybir.AluOpType.mult
    RELU = mybir.ActivationFunctionType.Relu
    EXP = mybir.ActivationFunctionType.Exp

    with tile.TileContext(nc) as tc:
        with (
            tc.tile_pool(name="singles", bufs=1) as singles,
            tc.tile_pool(name="xpool", bufs=XBUFS) as xpool,
            tc.tile_pool(name="ppool", bufs=PBUFS) as ppool,
            tc.tile_pool(name="ptpool", bufs=PBUFS) as ptpool,
            tc.tile_pool(name="small", bufs=6) as small,
            tc.tile_pool(name="ps_s", bufs=SBUFS, space=bass.MemorySpace.PSUM) as ps_s,
            tc.tile_pool(name="ps_t", bufs=2, space=bass.MemorySpace.PSUM) as ps_t,
            tc.tile_pool(name="ps_o", bufs=OBUFS, space=bass.MemorySpace.PSUM) as ps_o,
        ):
            # input loads split across the two HWDGE queues (SP + ACT)
            sb_WlT = singles.tile([128, 128], f16)
            nc.sync.dma_start(sb_WlT, WlT)
            sb_WrT = singles.tile([128, 128], f16)
            nc.scalar.dma_start(sb_WrT, WrT)
            sb_hT = singles.tile([128, N], f16)
            nc.sync.dma_start(sb_hT, hT)
            sb_WS = singles.tile([128, BLK, 32], f16)
            nc.scalar.dma_start(sb_WS, WS)
            sb_Wv = singles.tile([128, 128], f16)
            nc.scalar.dma_start(sb_Wv, Wv)
            # all 16 blocks' normalized outputs land here; 16 pre-dispatched
            # DMAs (2 half-ranges x 8 heads) extract the per-head diagonals
            stage16 = singles.tile([128, NBLK, 128], f16)

            # ---- prolog: projections (all-f16 matmuls, 1 cyc/col) ----
            g_ps = ps_s.tile([128, HALF], f32, tag="S")
            nc.tensor.matmul(g_ps, lhsT=sb_WlT, rhs=sb_hT[:, :HALF],
                             start=True, stop=True)
            sb_glT = singles.tile([128, HALF], f32)
            nc.vector.tensor_copy(sb_glT, g_ps)

            r_ps = ps_s.tile([128, N], f32, tag="S")
            nc.tensor.matmul(r_ps, lhsT=sb_WrT, rhs=sb_hT, start=True, stop=True)
            sb_grT = singles.tile([128, N], f16)
            nc.vector.tensor_copy(sb_grT, r_ps)

            # gr_cat[j, (h,d)] via XBAR transpose + ones column at 128;
            # chunks are padded to 256 so every transpose lands on a
            # 512B-aligned destination (the XBAR miscomputes otherwise)
            sb_grcat = singles.tile([128, 4, 256], f16)
            nc.vector.memset(sb_grcat[:, :, 128], 1.0)
            nc.sync.dma_start_transpose(sb_grcat[:, :, 0:128], sb_grT)

            # ---- main loop: 16 blocks of 16 token-rows ---------------
            # Tail pieces of block k are emitted inside block k+1's q loop at
            # staggered points so every in-order engine keeps streaming:
            #   q0:        exp (ACT) + 4 XBAR transpose dispatches (SP)
            #   TAIL_O_Q:  out-projection (PE reaches it ~1.8us in, after the
            #              XBAR transfers have surely landed)
            #   TAIL_R_Q:  reciprocal (DVE program reaches it even later)
            #   post-Wv:   normalize+relu (DVE)
            def make_tails(blk, S_ps):
                P = ppool.tile([128, N], bf16, tag="P")
                PT = ptpool.tile([128, 4, 128], bf16, tag="PT")
                T_ps = (
                    ps_t.tile([128, 4, 128], bf16, tag="T")
                    if PT_MODE == "eng" else None
                )
                O_ps = ps_o.tile([128, 129], f32, tag="O")
                Zi = small.tile([128, 1], f32, tag="Zi", name=f"Zi{blk}")

                def tail1a():
                    nc.scalar.activation(P, S_ps, EXP)
                    if PT_MODE == "xbar":
                        # one whole-tensor XBAR transpose: lands exactly in
                        # the chunked layout PT[:, c] = P[:, c*128:...].T
                        nc.sync.dma_start_transpose(PT, P)

                def tail1b():
                    # O rows keep the scrambled S order 32*(i2%4)+8*(i2//4)+h;
                    # the host-side unshard gather resolves it for free
                    for cch in range(4):
                        nc.tensor.matmul(
                            O_ps,
                            lhsT=PT[:, cch],
                            rhs=sb_grcat[:, cch, 0:129],
                            start=(cch == 0),
                            stop=(cch == 3),
                        )

                def tail2a():
                    nc.vector.reciprocal(Zi, O_ps[:, 128:129])

                def tail2b():
                    nc.vector.tensor_scalar(
                        stage16[:, blk], O_ps[:, 0:128], Zi, 0.0, MULT, MAX
                    )

                def tail_eng_pt():
                    # fallback: PE transpose + engine copies (PT_MODE=eng)
                    for cch in range(4):
                        nc.tensor.transpose(
                            T_ps[:, cch], P[:, cch * 128 : (cch + 1) * 128],
                            sb_id,
                        )
                    for cch in range(4):
                        if cch % 2 == 0:
                            nc.vector.tensor_copy(PT[:, cch], T_ps[:, cch])
                        else:
                            nc.scalar.copy(PT[:, cch], T_ps[:, cch])

                if PT_MODE == "eng":
                    def tail1a_eng():
                        nc.scalar.activation(P, S_ps, EXP)
                        tail_eng_pt()
                    return tail1a_eng, tail1b, tail2a, tail2b
                return tail1a, tail1b, tail2a, tail2b

            if PT_MODE == "eng":
                ident = nc.dram_tensor(
                    "ident", (128, 128), bf16, kind="ExternalInput"
                ).ap()
                sb_id = singles.tile([128, 128], bf16)
                nc.scalar.dma_start(sb_id, ident)

            # tails are pipelined two blocks deep: exp/transpose/out-proj of
            # block k run during block k+1, normalize (recip+relu) during
            # block k+2 — so the in-order DVE never waits on the out-proj
            pending = []
            for blk in range(NBLK):
                S_ps = ps_s.tile([128, N], f32, tag="S", name=f"S_{blk}")
                for q in range(BLK):
                    i = blk * BLK + q
                    X = xpool.tile([128, N], f16, tag="X")
                    gl_col = sb_glT[:, i : i + 1]
                    if q >= BLK - ACT_QS:
                        nc.scalar.activation(X, sb_grT, RELU, bias=gl_col,
                                             scale=1.0)
                    else:
                        nc.vector.tensor_scalar(X, sb_grT, gl_col, 0.0, ADD, MAX)
                    # S row for (i2=q, h) is 32*(q%4) + 8*(q//4) + h: four
                    # consecutive q's use the four 32-col PE array tiles
                    # (weights are zero outside their 8 columns, so q<4's
                    # start=True resets each 32-row group exactly once)
                    gq = q % 4
                    nc.tensor.matmul(
                        S_ps[32 * gq : 32 * gq + 32, :],
                        lhsT=sb_WS[:, q],
                        rhs=X,
                        start=(q < 4),
                        stop=False,
                        tile_position=(0, 32 * gq),
                        skip_group_check=True,
                    )
                    if q == 0 and pending:
                        pending[-1][0]()
                    if q == TAIL_O_Q and pending:
                        pending[-1][1]()
                    if q == TAIL_R_Q and len(pending) == 2:
                        pending[0][2]()
                nc.tensor.matmul(
                    S_ps,
                    lhsT=sb_Wv,
                    rhs=sb_grT,
                    start=False,
                    stop=True,
                    skip_group_check=True,
                )
                if len(pending) == 2:
                    pending.pop(0)[3]()
                if blk == NBLK // 2 + 2:
                    # first half of the staging tile is complete: stream it
                    # out while the second half computes (scalar ring so the
                    # per-block XBAR transposes on the SP ring never wait)
                    nc.scalar.dma_start(
                        out[:, : (NBLK // 2) * 128],
                        stage16[:, : NBLK // 2, :],
                    )
                pending.append(make_tails(blk, S_ps))
            # drain: block 14 still owes its normalize; block 15 everything
            pending[0][2](); pending[0][3]()
            t1a, t1b, t2a, t2b = pending[1]
            t1a(); t1b(); t2a(); t2b()
            nc.scalar.dma_start(
                out[:, (NBLK // 2) * 128 :], stage16[:, NBLK // 2 :, :]
            )

    nc.compile()
    return nc


@functools.lru_cache(maxsize=1)
def get_program():
    return build_program()


def host_prep(h, Wl, Wr, Wak):
    """Build per-core input maps (all numpy, no device work)."""
    h = np.asarray(h, dtype=np.float32)
    Wl = np.asarray(Wl, dtype=np.float32)
    Wr = np.asarray(Wr, dtype=np.float32)
    Wak = np.asarray(Wak, dtype=np.float32)

    hT_all = np.ascontiguousarray(h.transpose(0, 2, 1))          # (B, 128, N)
    WlT = np.ascontiguousarray(
        Wl.transpose(1, 0, 2).reshape(INPUT_DIM, N_HEADS * HEAD_DIM)
    ).astype(np.float16)
    WrT = np.ascontiguousarray(
        Wr.transpose(1, 0, 2).reshape(INPUT_DIM, N_HEADS * HEAD_DIM)
    ).astype(np.float16)

    # WS[q][(h,d), 8*(q//4)+h] = 0.8 * Wak[h]; S row = 32*(q%4)+8*(q//4)+h
    WS = np.zeros((128, BLK, 32), dtype=np.float32)
    for hh in range(N_HEADS):
        for q in range(BLK):
            WS[hh * 16 : hh * 16 + 16, q, (q // 4) * 8 + hh] = 0.8 * Wak[hh]
    # Wv adds 0.2*v_hj to every S row (same scrambled row order)
    Wv = np.zeros((128, 128), dtype=np.float32)
    for hh in range(N_HEADS):
        for i2 in range(BLK):
            row = 32 * (i2 % 4) + 8 * (i2 // 4) + hh
            Wv[hh * 16 : hh * 16 + 16, row] = 0.2 * Wak[hh]
    WS = WS.astype(np.float16)
    Wv = Wv.astype(np.float16)

    in_maps = []
    for c in range(N_CORES):
        b = c // 2
        i0 = (c % 2) * HALF
        hT = hT_all[b]
        # j-permutation: own token half first (queries are cols 0:256)
        hTc = np.concatenate(
            [hT[:, i0 : i0 + HALF], hT[:, HALF - i0 : N - i0]], axis=1
        ).astype(np.float16)
        m = {
            "hT": np.ascontiguousarray(hTc),
            "WlT": WlT,
            "WrT": WrT,
            "WS": WS,
            "Wv": Wv,
        }
        if PT_MODE == "eng":
            # transpose permutation: row 32*(i2%4)+8*(i2//4)+h -> col h*16+i2
            ident = np.zeros((128, 128), dtype=np.float32)
            for hh in range(N_HEADS):
                for i2 in range(BLK):
                    ident[32 * (i2 % 4) + 8 * (i2 // 4) + hh,
                          hh * 16 + i2] = 1.0
            m["ident"] = ident.astype(ml_dtypes.bfloat16)
        in_maps.append(m)
    return in_maps


def run_on_cores(in_maps, trace=False):
    from concourse.bass_utils import run_bass_kernel_spmd

    nc = get_program()
    return run_bass_kernel_spmd(
        nc, in_maps, core_ids=list(range(N_CORES)), trace=trace
    )


_I2 = np.arange(BLK)
_ROWS = 32 * (_I2 % 4)[:, None] + 8 * (_I2 // 4)[:, None] + np.arange(8)[None, :]


def _unshard(raw):
    """Gather the per-head diagonals out of the raw scrambled staging dump:
    out[blk*16+i2, h*16+d] = raw[32*(i2%4)+8*(i2//4)+h, blk, h*16+d]."""
    stage = np.asarray(raw, dtype=np.float32).reshape(128, NBLK, N_HEADS, 16)
    sel = stage[_ROWS]                        # [i2, h, blk, h2, d]
    hh = np.arange(N_HEADS)
    picked = sel[_I2[:, None], hh[None, :], :, hh[None, :], :]  # [i2, h, blk, d]
    return picked.transpose(2, 0, 1, 3).reshape(HALF, 128)


def kernel(h, mask, Wl, Wr, Wak):
    """Full-input / full-output entry point (mask is all-False by problem
    construction; masked-off attention is a no-op and is not computed)."""
    in_maps = host_prep(h, Wl, Wr, Wak)
    res = run_on_cores(in_maps, trace=False)
    full = np.empty((B, N, INPUT_DIM), dtype=np.float32)
    for c in range(N_CORES):
        b = c // 2
        i0 = (c % 2) * HALF
        full[b, i0 : i0 + HALF] = _unshard(res.results[c]["out"])
    return full
